# revision 26
# baseline (speedup 1.0000x reference)
"""Causal self-attention (B=1, T=4096, C=768, H=12, D=64) on 8 NeuronCores.

Sharding: tensor-parallel over heads. Cores 0-3 take head pairs
(0,1),(2,3),(4,5),(6,7); cores 4-7 take heads 8,9,10,11 plus a zero-weight
dummy head (uniform SPMD program). Each core computes, for its 2 local heads:

  qkT = [Wq/8 | Wk]^T @ x^T + b      -> [128, T]   (q rows 0:64, k rows 64:128)
  vT  = [Wv_a | Wv_b]^T @ x^T        -> [128, T], PE-transposed to v [T, 64+1]
                                        (ones column folded in for softmax sums)
  s^T[k,q] = k.q/8 (+ causal tri mask added via a bf16 matmul on diag blocks)
  p^T = exp(s^T)                     (no max-subtraction: |logits| <~ 4)
  yT_raw[65, q] = [v|1]^T @ p^T      (row 64 = softmax denominators)
  yT = yT_raw[0:64] * bcast(1/denom)
  out_partial[q, 768] = yT(2 heads)^T @ Wp_rows

The denominator row is PE-transposed to a column so the DVE reciprocal runs
parallel across lanes, then transposed back and broadcast with K=1 matmuls.

Host sums the 8 partials and adds b_proj + b_attn[v] @ w_proj (v-bias is
linear through attention since softmax rows sum to 1).

Matmul-feeding tensors are float32r (FP22 multiply, fp32 accumulate): full
PE rate, ~1e-4 relative error.
"""
import numpy as np
from contextlib import ExitStack

import concourse.bass as bass
import concourse.mybir as mybir
import concourse.tile as tile
from concourse import bacc
from concourse.bass import ts
from concourse.bass_utils import run_bass_kernel_spmd

try:
    import ml_dtypes
    ml_bf16 = ml_dtypes.bfloat16
except ImportError:  # pragma: no cover
    ml_bf16 = np.float32

F32 = mybir.dt.float32
F32R = mybir.dt.float32r
BF16 = mybir.dt.bfloat16
EXP = mybir.ActivationFunctionType.Exp

T, C, H, D = 4096, 768, 12, 64
NH = 2                 # local heads per core
KC = C // 128          # 6 contraction chunks of 128
TQ = 512               # q supertile width
NJ = T // TQ           # 8 supertiles
NT = T // 128          # 32 k-tiles
CH = 2                 # k-tiles per exp chunk (2 PSUM banks, double buffered)
NEG = -60.0            # additive mask value (exp(-60) ~ 0)

_CACHE = {}


def build_program():
    nc = bacc.Bacc()
    xT = nc.dram_tensor("xT", [C, T], F32R, kind="ExternalInput")
    wqk = nc.dram_tensor("wqk", [NH, C, 128], F32R, kind="ExternalInput")
    bqk = nc.dram_tensor("bqk", [NH, 128], F32, kind="ExternalInput")
    wv = nc.dram_tensor("wv", [C, NH * 64], F32R, kind="ExternalInput")
    wp = nc.dram_tensor("wp", [NH * 64, C], F32R, kind="ExternalInput")
    tri = nc.dram_tensor("tri", [128, 128], BF16, kind="ExternalInput")
    identb = nc.dram_tensor("identb", [128, 128], BF16, kind="ExternalInput")
    ident = nc.dram_tensor("ident", [128, 128], F32R, kind="ExternalInput")
    onesd = nc.dram_tensor("onesd", [NT * NH * 65], F32R, kind="ExternalInput")
    out = nc.dram_tensor("out", [T, C], F32, kind="ExternalOutput")

    with ExitStack() as ctx:
        tc = ctx.enter_context(tile.TileContext(nc))
        singles = ctx.enter_context(tc.tile_pool(name="singles", bufs=1))
        ring = ctx.enter_context(tc.tile_pool(name="ring", bufs=12))
        vring = ctx.enter_context(tc.tile_pool(name="vring", bufs=4))
        sb_p = ctx.enter_context(tc.tile_pool(name="sb_p", bufs=4))
        sb_y = ctx.enter_context(tc.tile_pool(name="sb_y", bufs=2))
        sb_r = ctx.enter_context(tc.tile_pool(name="sb_r", bufs=2))
        sb_o = ctx.enter_context(tc.tile_pool(name="sb_o", bufs=3))
        dscr = ctx.enter_context(tc.tile_pool(name="dscr", bufs=2, space="DRAM"))
        ps_qk_cm = tc.tile_pool(name="ps_qk", bufs=3, space="PSUM")
        ps_qk = ps_qk_cm.__enter__()

        # ---- constants / weights (small, loaded first) ----
        wqk_sb = singles.tile([128, NH, KC, 128], F32R)
        nc.sync.dma_start(
            wqk_sb, wqk.rearrange("h (kc p) m -> p h kc m", p=128))
        bqk_sb = singles.tile([128, NH], F32)
        nc.sync.dma_start(bqk_sb, bqk.rearrange("h p -> p h"))
        wv_sb = singles.tile([128, KC, NH * 64], F32R)
        nc.sync.dma_start(wv_sb, wv.rearrange("(kc p) m -> p kc m", p=128))
        wp_sb = singles.tile([128, C], F32R)
        tri_sb = singles.tile([128, 128], BF16)
        nc.sync.dma_start(tri_sb, tri[:, :])
        identb_sb = singles.tile([128, 128], BF16)
        nc.sync.dma_start(identb_sb, identb[:, :])
        ident_sb = singles.tile([128, 128], F32R)
        nc.sync.dma_start(ident_sb, ident[:, :])
        v_sb = singles.tile([128, NT * NH * 65], F32R)

        # persistent per-head state
        qkT = []
        for h in range(NH):
            qkT_h = singles.tile([128, T], F32R, tag=f"qkT{h}")
            qkT.append(qkT_h)
        k0 = singles.tile([64, T], F32R)        # head0 k, base 0
        q1t = singles.tile([128, T], F32R)      # head1 q relocated to base 64
        yfin = singles.tile([128, T], F32R)     # normalized yT, both heads

        def vslot(i, h):
            return (i * NH + h) * 65

        def qkv_step(tc_i):
            """Load x column slice, compute qkT/vT chunks for both heads,
            transpose v k-tiles 4*tc_i..4*tc_i+3, relocate k0/q1."""
            xs = []
            for kc in range(KC):
                x_sl = ring.tile([128, TQ], F32R, tag="xr")
                nc.sync.dma_start(x_sl, xT[ts(kc, 128), ts(tc_i, TQ)])
                xs.append(x_sl)
            for h in range(NH):
                ps = ps_qk.tile([128, TQ], F32, tag="qk")
                for kc in range(KC):
                    nc.tensor.matmul(
                        ps, lhsT=wqk_sb[:, h, kc, :], rhs=xs[kc],
                        start=(kc == 0), stop=(kc == KC - 1))
                nc.vector.tensor_scalar_add(
                    qkT[h][:, ts(tc_i, TQ)], ps, bqk_sb[:, h : h + 1])
            nc.sync.dma_start(k0[:, ts(tc_i, TQ)], qkT[0][64:128, ts(tc_i, TQ)])
            nc.sync.dma_start(q1t[64:128, ts(tc_i, TQ)],
                              qkT[1][0:64, ts(tc_i, TQ)])
            pv_ = ps_qk.tile([128, TQ], F32, tag="qk")
            for kc in range(KC):
                nc.tensor.matmul(
                    pv_, lhsT=wv_sb[:, kc, :], rhs=xs[kc],
                    start=(kc == 0), stop=(kc == KC - 1))
            vt_c = vring.tile([128, TQ], F32R, tag="vt")
            nc.vector.tensor_copy(vt_c, pv_)
            for h in range(NH):
                for il in range(4):
                    i = 4 * tc_i + il
                    tp = ps_qk.tile([128, 64], F32R, tag="qk")
                    nc.tensor.transpose(
                        tp, vt_c[ts(h, 64), ts(il, 128)],
                        ident_sb[ts(h, 64), ts(h, 64)])
                    nc.vector.tensor_copy(
                        v_sb[:, vslot(i, h) : vslot(i, h) + 64], tp)

        def att_gen(h, J):
            nkt = 4 * J + 4
            chunks = [list(range(nkt))[i : i + CH] for i in range(0, nkt, CH)]
            yt = ps_yt.tile([128, TQ], F32, tag="yt")
            state = {"first": True}

            def emit_s(ch_tiles):
                st = ps_s.tile([128, CH * TQ], F32, tag="st")
                for j, i in enumerate(ch_tiles):
                    d = i - 4 * J
                    if h == 0:
                        nc.tensor.matmul(
                            st[:, ts(j, TQ)], lhsT=k0[:, ts(i, 128)],
                            rhs=qkT[0][0:64, ts(J, TQ)],
                            start=True, stop=(d < 0))
                    else:
                        nc.tensor.matmul(
                            st[:, ts(j, TQ)], lhsT=qkT[1][64:128, ts(i, 128)],
                            rhs=q1t[64:128, ts(J, TQ)],
                            start=True, stop=(d < 0))
                    if d >= 0:
                        nc.tensor.matmul(
                            st[:, j * TQ + d * 128 : j * TQ + (d + 1) * 128],
                            lhsT=tri_sb, rhs=identb_sb,
                            start=False, stop=True, skip_group_check=True)
                pt = sb_p.tile([128, CH * TQ], F32R, tag="pt")
                n = len(ch_tiles) * TQ
                nc.scalar.activation(pt[:, :n], st[:, :n], EXP)
                return pt

            def emit_pv(ch_tiles, pt):
                for j, i in enumerate(ch_tiles):
                    d = i - 4 * J
                    q0 = d * 128 if d > 0 else 0
                    nc.tensor.matmul(
                        yt[0:65, q0:TQ],
                        lhsT=v_sb[:, vslot(i, h) : vslot(i, h) + 65],
                        rhs=pt[:, j * TQ + q0 : (j + 1) * TQ],
                        start=state["first"], stop=(i == nkt - 1),
                        skip_group_check=True)
                    state["first"] = False

            pts = []
            for ci in range(len(chunks) + 1):
                if ci < len(chunks):
                    pts.append(emit_s(chunks[ci]))
                if ci >= 1:
                    emit_pv(chunks[ci - 1], pts[ci - 1])
                yield

            # normalize: yfin[h] = yt[0:64] / yt[64] (DMA rearranges + recip)
            yraw = sb_y.tile([128, TQ], F32R, tag="sb_y")
            nc.vector.tensor_copy(yraw[0:65], yt[0:65])
            den_d = dscr.tile([TQ], F32R, tag="den_d")
            nc.sync.dma_start(den_d[:], yraw[64:65, :])
            dc = sb_r.tile([128, 4], F32R, tag="dc")
            nc.sync.dma_start(dc, den_d[:].rearrange("(i q) -> q i", i=4))
            rec_c = sb_r.tile([128, 4], F32R, tag="rec_c")
            with nc.allow_low_precision(reason="fp32r for PE"):
                nc.vector.reciprocal(rec_c, dc)
            rr_d = dscr.tile([TQ], F32R, tag="rr_d")
            nc.sync.dma_start(rr_d[:].rearrange("(i q) -> q i", i=4), rec_c)
            bc = sb_r.tile([64, TQ], F32R, tag="bc")
            nc.sync.dma_start(bc, rr_d[:].partition_broadcast(64))
            if h == 0:
                nc.vector.tensor_mul(yfin[0:64, ts(J, TQ)], yraw[0:64], bc)
            else:
                ytmp = sb_r.tile([64, TQ], F32R, tag="ytmp")
                nc.vector.tensor_mul(ytmp, yraw[0:64], bc)
                nc.sync.dma_start(yfin[64:128, ts(J, TQ)], ytmp)

        def proj_step(J):
            for qt in range(4):
                q0 = J * TQ + qt * 128
                ob = sb_o.tile([128, C], F32, tag="ob")
                pp = ps_pr.tile([128, 512], F32, tag="pp")
                nc.tensor.matmul(pp, lhsT=yfin[:, q0 : q0 + 128],
                                 rhs=wp_sb[:, 0:512], start=True, stop=True)
                nc.vector.tensor_copy(ob[:, 0:512], pp)
                pp2 = ps_pr.tile([128, 256], F32, tag="pp")
                nc.tensor.matmul(pp2, lhsT=yfin[:, q0 : q0 + 128],
                                 rhs=wp_sb[:, 512:768], start=True, stop=True)
                nc.vector.tensor_copy(ob[:, 512:768], pp2)
                nc.sync.dma_start(out[q0 : q0 + 128, :], ob)

        for t in range(NJ):
            qkv_step(t)
            if t == 0:
                nc.sync.dma_start(wp_sb, wp[:, :])
                ones_view = bass.AP(
                    tensor=v_sb.tensor, offset=v_sb.offset + 64,
                    ap=[list(p) for p in v_sb.ap[:1]] + [[65, NT * NH]])
                nc.sync.dma_start(
                    ones_view, onesd[:][0 : NT * NH].partition_broadcast(128))
        ps_qk_cm.__exit__(None, None, None)
        ps_s = ctx.enter_context(tc.tile_pool(name="ps_s", bufs=2, space="PSUM"))
        ps_yt = ctx.enter_context(tc.tile_pool(name="ps_yt", bufs=2, space="PSUM"))
        ps_pr = ctx.enter_context(tc.tile_pool(name="ps_pr", bufs=2, space="PSUM"))
        done = set()
        for s in range(NJ):
            gens = [att_gen(0, s), att_gen(1, NJ - 1 - s)]
            while gens:
                for g in list(gens):
                    try:
                        next(g)
                    except StopIteration:
                        gens.remove(g)
            for J in range(NJ):
                if J not in done and J <= s and (NJ - 1 - J) <= s:
                    proj_step(J)
                    done.add(J)

    if not nc.is_finalized():
        nc.finalize()
    return nc


def _make_inputs(x, w_attn, b_attn, w_proj):
    """Build the 8 per-core input maps from full inputs."""
    xTc = np.ascontiguousarray(x.reshape(T, C).T).astype(np.float32)
    tri_np = np.where(np.arange(128)[:, None] >= np.arange(128)[None, :],
                      0.0, NEG).astype(ml_bf16)
    identb_np = np.eye(128, dtype=np.float32).astype(ml_bf16)
    ident_np = np.eye(128, dtype=np.float32)
    onesd_np = np.ones((NT * NH * 65,), np.float32)

    heads_per_core = [(0, 1), (2, 3), (4, 5), (6, 7),
                      (8, None), (9, None), (10, None), (11, None)]
    in_maps = []
    for heads in heads_per_core:
        wqk_np = np.zeros((NH, C, 128), np.float32)
        bqk_np = np.zeros((NH, 128), np.float32)
        wv_np = np.zeros((C, NH * 64), np.float32)
        wp_np = np.zeros((NH * 64, C), np.float32)
        for hi, h in enumerate(heads):
            if h is None:
                continue
            qc, kc_, vc = h * 64, C + h * 64, 2 * C + h * 64
            wqk_np[hi, :, 0:64] = w_attn[:, qc : qc + 64] * 0.125
            wqk_np[hi, :, 64:128] = w_attn[:, kc_ : kc_ + 64]
            bqk_np[hi, 0:64] = b_attn[qc : qc + 64] * 0.125
            bqk_np[hi, 64:128] = b_attn[kc_ : kc_ + 64]
            wv_np[:, hi * 64 : (hi + 1) * 64] = w_attn[:, vc : vc + 64]
            wp_np[hi * 64 : (hi + 1) * 64, :] = w_proj[h * 64 : (h + 1) * 64, :]
        in_maps.append({
            "onesd": onesd_np,
            "xT": xTc, "wqk": wqk_np, "bqk": bqk_np, "wv": wv_np,
            "wp": wp_np, "tri": tri_np, "identb": identb_np,
            "ident": ident_np,
        })
    return in_maps


def kernel(x, w_attn, b_attn, w_proj, b_proj, _trace=False):
    x = np.asarray(x, np.float32)
    w_attn = np.asarray(w_attn, np.float32)
    b_attn = np.asarray(b_attn, np.float32)
    w_proj = np.asarray(w_proj, np.float32)
    b_proj = np.asarray(b_proj, np.float32)

    if "nc" not in _CACHE:
        _CACHE["nc"] = build_program()
    nc = _CACHE["nc"]
    in_maps = _make_inputs(x, w_attn, b_attn, w_proj)
    res = run_bass_kernel_spmd(nc, in_maps, core_ids=list(range(8)),
                               trace=_trace)
    total = np.zeros((T, C), np.float32)
    for c in range(8):
        total += res.results[c]["out"]
    total += b_proj[None, :] + (b_attn[2 * C :] @ w_proj)[None, :]
    if _trace:
        _CACHE["last_result"] = res
    return total.reshape(1, T, C)
